# revision 41
# baseline (speedup 1.0000x reference)
"""ComplexCrossAttention Trainium2 kernel: 8 cores = DP(batch=2) x TP(head-groups=4).

Each core (b = core//4, g = core%4) handles batch b and heads 4g..4g+3; the
host adds the four per-group partial Wo outputs (the hint's all-reduce).

All matmul operands are fp16 (same 1 col/cycle PE rate as bf16 on TRN2, 3 extra
mantissa bits of precision; fp8 DoubleRow measured at only 2x-per-pass here, so
residual-fp8 loses to 16-bit).  Complex arithmetic is folded into the matmul
contraction: the j=0/j=1 planes of each packed operand carry the (w1, x_r) and
(w2, x_i) terms of  [real;imag] = w1^T x_r + w2^T x_i,  with w1 = [Wr|Wi],
w2 = [-Wi|Wr] column blocks per head.

Schedule: K proj, V proj, then per q-tile qt: attention(qt) is followed by
Q proj(qt+1) BEFORE O proj(qt), so the projection fills the softmax-tail
latency (dn -> reciprocal -> broadcast -> OT) of the last head.

Scores live transposed [k, q] so the mask is a per-partition activation bias.
Softmax denominators: e-tiles tree-summed on DVE (7 adds), ONE ones-matmul
[128,8]^T @ esum -> dn[8,512], reciprocal_approx_fast on [8,512], and a K=8
0.125-matmul broadcasting 1/dn to [128,512] PSUM.  This replaces the previous
128 full-size dn matmuls and 16 x 3.4us full DVE reciprocals.

Outputs are written as fp16 partials (summed on host).  Bias terms are all
zero in this problem and are folded out.
"""

import numpy as np
import ml_dtypes

import concourse.bacc as bacc
import concourse.mybir as mybir
import concourse.tile as tile
from concourse.bass_utils import run_bass_kernel_spmd

F32 = mybir.dt.float32
F16 = mybir.dt.float16
NP16 = np.float16

B, S, Lc = 2, 2048, 1024
F, Dc, H = 1024, 768, 16
HD = 64
NCORES = 8
TPG = 4            # head-groups (TP degree per batch)
FS = F // TPG      # 256 features per core
HL = 4             # heads per core
NQ, QTS = 4, 512   # q tiles
NKT = 8            # k tiles of 128 (Lc)
NFIN = 8           # f_in 128-chunks (Q proj contraction, per component)
NDC = 6            # Dc 128-chunks (K/V proj contraction, per component)
SCALE = 1.0 / 8.0  # 1/sqrt(HD)

_CACHE = {}

EXP = mybir.ActivationFunctionType.Exp


def _build_nc():
    nc = bacc.Bacc()
    dt = mybir.dt

    d = {}
    for nm, shape in (
        ("x", [128, NQ, NFIN, 2, QTS]),
        ("wq", [128, NFIN, 2, 512]),
        ("ct", [128, NDC, 2, Lc]),
        ("wk", [128, NDC, 2, 512]),
        ("wv", [128, NDC, 2, 512]),
        ("wo1", [128, 2, 2, F]),
        ("wo2", [128, 2, 2, F]),
    ):
        d[nm] = nc.dram_tensor(nm, shape, dt.float16, kind="ExternalInput")
    d["mb"] = nc.dram_tensor("mb", [128, NKT], dt.float32, kind="ExternalInput")
    yr_d = nc.dram_tensor("yr", [S, F], dt.float16, kind="ExternalOutput")
    yi_d = nc.dram_tensor("yi", [S, F], dt.float16, kind="ExternalOutput")

    with tile.TileContext(nc) as tc:
        with (
            tc.tile_pool(name="res", bufs=1) as res,
            tc.tile_pool(name="qx", bufs=8) as qxp,       # QX per (qt,h)
            tc.tile_pool(name="ep", bufs=8) as ep,        # exp(scores)
            tc.tile_pool(name="esp", bufs=8) as esp,      # e tree sums
            tc.tile_pool(name="oth", bufs=2) as othp,     # OT per qt
            tc.tile_pool(name="rcs", bufs=2) as rcs,      # small rec tiles
            tc.tile_pool(name="rcb", bufs=2) as rcbp,     # rec broadcast sbuf
            tc.tile_pool(name="ys", bufs=3) as ys,        # y staging
            tc.tile_pool(name="pp", bufs=3, space="PSUM") as pp,    # projections
            tc.tile_pool(name="sp", bufs=2, space="PSUM") as spp,   # scores
            tc.tile_pool(name="ap", bufs=2, space="PSUM") as avp,   # attn @ V
            tc.tile_pool(name="dn", bufs=1, space="PSUM") as dnp,   # dn [8,512]
        ):
            def rtile(shape, dtype, tag):
                return res.tile(shape, dtype, tag=tag, name=tag)

            # ---- input tiles + DMA (3 queues; order = consumption order) --
            sb = {}

            def load(name, shape, queue):
                t = rtile(shape, F16, name)
                queue.dma_start(t[:], d[name][:])
                sb[name] = t

            # Aggregate DMA bandwidth is only ~140-150 GB/s/core, so the
            # schedule is arrival-order-critical.  Keep >=8KB/partition
            # descriptors (small strided descriptors measured ~5x slower).
            # x is q-tile-major so Q(qt) gates only on its own 2MB slice.
            ct = rtile([128, NDC, 2, Lc], F16, "ct")
            xt = rtile([128, NQ, NFIN, 2, QTS], F16, "x")
            wk = rtile([128, NDC, 2, 512], F16, "wk")
            wv = rtile([128, NDC, 2, 512], F16, "wv")
            nc.scalar.dma_start(wk[:, 0:3], d["wk"][:, 0:3])
            nc.scalar.dma_start(wv[:, 0:3], d["wv"][:, 0:3])
            nc.scalar.dma_start(wk[:, 3:6], d["wk"][:, 3:6])
            nc.scalar.dma_start(wv[:, 3:6], d["wv"][:, 3:6])
            sb["wk"], sb["wv"] = wk, wv
            for c in range(NDC):
                (nc.sync if c % 2 == 0 else nc.gpsimd).dma_start(
                    ct[:, c:c + 1], d["ct"][:, c:c + 1])
            sb["ct"] = ct
            nc.sync.dma_start(xt[:, 0, 0:4], d["x"][:, 0, 0:4])
            nc.gpsimd.dma_start(xt[:, 0, 4:8], d["x"][:, 0, 4:8])
            nc.gpsimd.dma_start(xt[:, 1], d["x"][:, 1])
            nc.sync.dma_start(xt[:, 2], d["x"][:, 2])
            nc.gpsimd.dma_start(xt[:, 3], d["x"][:, 3])
            sb["x"] = xt
            load("wq", [128, NFIN, 2, 512], nc.scalar)
            mb = rtile([128, NKT], F32, "mb")
            nc.scalar.dma_start(mb[:], d["mb"][:])
            load("wo1", [128, 2, 2, F], nc.scalar)
            load("wo2", [128, 2, 2, F], nc.scalar)

            ones_dn = rtile([128, 8], F16, "ones_dn")
            nc.vector.memset(ones_dn[:], 1.0)
            bc_w = rtile([8, 128], F16, "bc_w")
            nc.vector.memset(bc_w[:], 0.125)

            KX = {h: rtile([128, Lc], F16, f"kx{h}") for h in range(HL)}
            Vsb = {kt: rtile([128, 512], F16, f"v{kt}") for kt in range(NKT)}

            def proj(ps, wname, xname, nch, lsl, rsl):
                last = nch * 2 - 1
                for c in range(nch):
                    for j in range(2):
                        nc.tensor.matmul(
                            ps, sb[wname][:, c, j, lsl], sb[xname][:, c, j, rsl],
                            start=(c == 0 and j == 0), stop=(c * 2 + j == last),
                        )

            def q_proj(qt):
                qx = {}
                for h in range(HL):
                    ps = pp.tile([128, 512], F32, tag="pp", name="pp")
                    hsl = slice(h * 128, (h + 1) * 128)
                    for c in range(NFIN):
                        for j in range(2):
                            nc.tensor.matmul(
                                ps[:], sb["wq"][:, c, j, hsl],
                                sb["x"][:, qt, c, j, :],
                                start=(c == 0 and j == 0),
                                stop=(c == NFIN - 1 and j == 1),
                            )
                    t = qxp.tile([128, 512], F16, tag="qx", name="qx")
                    nc.vector.tensor_copy(t[:], ps[:])
                    qx[h] = t
                return qx

            def attention(qt, qx):
                oth = othp.tile([128, HL, 512], F16, tag="oth", name="oth")
                for h in range(HL):
                    av = avp.tile([128, 512], F32, tag="av", name="av")
                    e_t, es_t = {}, []

                    def tree_add(a, b):
                        es = esp.tile([128, 512], F16, tag="es", name="es")
                        nc.vector.tensor_add(es[:], a[:], b[:])
                        return es

                    def scores_exp(kt):
                        ksl = slice(kt * 128, (kt + 1) * 128)
                        sps = spp.tile([128, 512], F32, tag="sp", name="sp")
                        nc.tensor.matmul(sps[:], KX[h][:, ksl], qx[h][:],
                                         start=True, stop=True)
                        e = ep.tile([128, 512], F16, tag="e", name="e")
                        nc.scalar.activation(e[:], sps[:], EXP,
                                             bias=mb[:, kt:kt + 1], scale=SCALE)
                        e_t[kt] = e
                        if kt % 2 == 1:
                            es_t.append(tree_add(e_t[kt - 1], e))
                            if kt % 4 == 3:
                                es_t.append(tree_add(es_t[-2], es_t[-1]))
                            if kt == 7:
                                es_t.append(tree_add(es_t[2], es_t[5]))

                    def av_mm(kt):
                        nc.tensor.matmul(av[:], Vsb[kt][:, h * 128:(h + 1) * 128],
                                         e_t[kt][:], start=(kt == 0),
                                         stop=(kt == NKT - 1))

                    # scores/exp lead av by one k-tile
                    for kt in range(NKT + 1):
                        if kt < NKT:
                            scores_exp(kt)
                        if kt > 0:
                            av_mm(kt - 1)

                    # dn bank is allocated full-height so pass A of the K/V
                    # projections can reuse it as an 8th accumulation group
                    dn = dnp.tile([128, 512], F32, tag="dn", name="dn")
                    nc.tensor.matmul(dn[0:8, :], ones_dn[:], es_t[-1][:],
                                     start=True, stop=True)
                    rec32 = rcs.tile([8, 512], F32, tag="r32", name="r32")
                    nc.vector.reciprocal_approx_fast(rec32[:], dn[0:8, :])
                    rec16 = rcs.tile([8, 512], F16, tag="r16", name="r16")
                    nc.vector.tensor_copy(rec16[:], rec32[:])
                    rbs = rcbp.tile([128, 512], F16, tag="rbs", name="rbs")
                    if qt == NQ - 1 and h >= 2:
                        # O3 starts right after att3 (no Q-proj interleave
                        # left); the last heads' 1/dn broadcast takes the
                        # lower-latency PE route to unblock its head-pair
                        rb = pp.tile([128, 512], F32, tag="pp", name="pp")
                        nc.tensor.matmul(rb[:], bc_w[:], rec16[:],
                                         start=True, stop=True)
                        nc.scalar.copy(rbs[:], rb[:])
                    else:
                        nc.gpsimd.partition_broadcast(rbs[:], rec16[0:1, :])
                    nc.vector.tensor_mul(oth[:, h, :], av[:], rbs[:])
                return oth

            def o_proj(qt, oth):
                for qi in range(4):
                    isl = slice(qi * 128, (qi + 1) * 128)
                    for kind, dram in (("wo1", yr_d), ("wo2", yi_d)):
                        st = ys.tile([128, F], F16, tag="y", name="y")
                        q0 = qt * QTS + qi * 128
                        last = qt == NQ - 1 and qi == 3
                        for fo in range(2):
                            fsl = slice(fo * 512, (fo + 1) * 512)
                            ps = pp.tile([128, 512], F32, tag="pp", name="pp")
                            for hp in range(2):
                                for j in range(2):
                                    nc.tensor.matmul(
                                        ps[:], oth[:, 2 * hp + j, isl],
                                        sb[kind][:, hp, j, fsl],
                                        start=(hp == 0 and j == 0),
                                        stop=(hp == 1 and j == 1),
                                    )
                            if (qi + fo) % 2 == 0:
                                nc.scalar.copy(st[:, fsl], ps[:])
                            else:
                                nc.vector.tensor_copy(st[:, fsl], ps[:])
                            if last:
                                # final row: per-half DMAs on both queues so
                                # the tail transfer halves and parallelizes
                                (nc.sync if fo == 0 else nc.gpsimd).dma_start(
                                    dram[q0:q0 + 128, fsl], st[:, fsl])
                        if not last:
                            (nc.sync if kind == "wo1" else nc.gpsimd).dma_start(
                                dram[q0:q0 + 128, :], st[:])

            # ---- K + V projections, two passes ---------------------------
            # Pass A: 3 K-groups + 4 V-groups held in 7 open PSUM banks,
            # chunk-outer, so matmuls consume each ct chunk DMA as it lands.
            # Pass B: the remaining 5 K + 4 V groups at full speed once ct
            # is resident.  This hides nearly all of the ct transfer.
            def k_mm(ps, kq, h, c, j, start, stop):
                nc.tensor.matmul(
                    ps, sb["wk"][:, c, j, h * 128:(h + 1) * 128],
                    sb["ct"][:, c, j, kq * 512:(kq + 1) * 512],
                    start=start, stop=stop)

            def v_mm(ps, kt, c, j, start, stop):
                nc.tensor.matmul(
                    ps, sb["ct"][:, c, j, kt * 128:(kt + 1) * 128],
                    sb["wv"][:, c, j, :], start=start, stop=stop)

            ka = [(0, 0), (0, 1), (0, 2)]
            va = [0, 1, 2, 3, 4]
            apools = [(pp, "pp"), (pp, "pp"), (pp, "pp"), (spp, "sp"),
                      (spp, "sp"), (avp, "av"), (avp, "av"), (dnp, "dn")]
            aps = {}
            for (pool, tag), key in zip(apools, ka + va):
                aps[key] = pool.tile([128, 512], F32, tag=tag, name="kv")
            for c in range(NDC):
                for j in range(2):
                    first, last = (c == 0 and j == 0), (c == NDC - 1 and j == 1)
                    for kq, h in ka:
                        k_mm(aps[(kq, h)][:], kq, h, c, j, first, last)
                    for kt in va:
                        v_mm(aps[kt][:], kt, c, j, first, last)
            for kq, h in ka:
                nc.vector.tensor_copy(KX[h][:, kq * 512:(kq + 1) * 512],
                                      aps[(kq, h)][:])
            for kt in va:
                nc.vector.tensor_copy(Vsb[kt][:], aps[kt][:])

            for kq, h in [(0, 3), (1, 0), (1, 1), (1, 2), (1, 3)]:
                ps = pp.tile([128, 512], F32, tag="pp", name="pp")
                for c in range(NDC):
                    for j in range(2):
                        k_mm(ps[:], kq, h, c, j, c == 0 and j == 0,
                             c == NDC - 1 and j == 1)
                nc.vector.tensor_copy(KX[h][:, kq * 512:(kq + 1) * 512], ps[:])
            for kt in range(5, NKT):
                ps = pp.tile([128, 512], F32, tag="pp", name="pp")
                for c in range(NDC):
                    for j in range(2):
                        v_mm(ps[:], kt, c, j, c == 0 and j == 0,
                             c == NDC - 1 and j == 1)
                nc.vector.tensor_copy(Vsb[kt][:], ps[:])

            # ---- pipelined per-q-tile: att(qt) -> Q(qt+1) -> O(qt) --------
            qx = q_proj(0)
            pend = None  # (qt, oth) awaiting O projection
            for qt in range(NQ):
                oth = attention(qt, qx)
                if qt + 1 < NQ:
                    qx = q_proj(qt + 1)
                if pend is not None:
                    o_proj(*pend)
                pend = (qt, oth)
            o_proj(*pend)

    nc.compile()
    return nc


def _prep_in_maps(inputs):
    f32 = np.float32
    x_r, x_i = np.asarray(inputs["x_r"], f32), np.asarray(inputs["x_i"], f32)
    ctx_r, ctx_i = np.asarray(inputs["ctx_r"], f32), np.asarray(inputs["ctx_i"], f32)
    mask = np.asarray(inputs["mask"], f32)
    W = {k: np.asarray(inputs[k], f32) for k in
         ("Wqr", "Wqi", "Wkr", "Wki", "Wvr", "Wvi", "Wor", "Woi")}

    def pack_moving(ar, ai, nch, n):
        """[n, nch*128] pair -> [128, nch, 2, n] fp16."""
        out = np.empty((128, nch, 2, n), NP16)
        out[:, :, 0, :] = ar.T.reshape(nch, 128, n).transpose(1, 0, 2)
        out[:, :, 1, :] = ai.T.reshape(nch, 128, n).transpose(1, 0, 2)
        return out

    per_batch = {}
    for b in range(B):
        xp = pack_moving(x_r[b], x_i[b], NFIN, S)  # [128, 8, 2, 2048]
        xp = np.ascontiguousarray(
            xp.reshape(128, NFIN, 2, NQ, QTS).transpose(0, 3, 1, 2, 4))
        per_batch[b] = {
            "x": xp,
            "ct": pack_moving(ctx_r[b], ctx_i[b], NDC, Lc),
            "mb": np.ascontiguousarray(
                ((1.0 - mask[b]) * -1e9).astype(f32).reshape(NKT, 128).T),
        }

    def merge_cols(Wr, Wi, g):
        """[Din, F] pair -> w1 = [Wr_h|Wi_h], w2 = [-Wi_h|Wr_h] col-blocks."""
        din = Wr.shape[0]
        w1 = np.empty((din, HL * 128), f32)
        w2 = np.empty((din, HL * 128), f32)
        for h in range(HL):
            cs = slice(g * FS + h * HD, g * FS + (h + 1) * HD)
            w1[:, h * 128:h * 128 + 64] = Wr[:, cs]
            w1[:, h * 128 + 64:(h + 1) * 128] = Wi[:, cs]
            w2[:, h * 128:h * 128 + 64] = -Wi[:, cs]
            w2[:, h * 128 + 64:(h + 1) * 128] = Wr[:, cs]
        return w1, w2

    def pack_w(w1, w2, nch):
        out = np.empty((128, nch, 2, 512), NP16)
        out[:, :, 0, :] = w1.reshape(nch, 128, 512).transpose(1, 0, 2)
        out[:, :, 1, :] = w2.reshape(nch, 128, 512).transpose(1, 0, 2)
        return out

    in_maps = []
    for core in range(NCORES):
        b, g = core // TPG, core % TPG
        m = dict(per_batch[b])
        for pre, wr, wi, nch in (("wq", "Wqr", "Wqi", NFIN),
                                 ("wk", "Wkr", "Wki", NDC),
                                 ("wv", "Wvr", "Wvi", NDC)):
            m[pre] = pack_w(*merge_cols(W[wr], W[wi], g), nch)
        # Wo rows in the merged [out_r(64); out_i(64)] layout, head-pair packed
        wo1 = np.empty((128, 2, 2, F), NP16)
        wo2 = np.empty((128, 2, 2, F), NP16)
        for h in range(HL):
            rs = slice(g * FS + h * HD, g * FS + (h + 1) * HD)
            hp, j = h // 2, h % 2
            wo1[:64, hp, j, :] = W["Wor"][rs]
            wo1[64:, hp, j, :] = -W["Woi"][rs]
            wo2[:64, hp, j, :] = W["Woi"][rs]
            wo2[64:, hp, j, :] = W["Wor"][rs]
        m["wo1"], m["wo2"] = wo1, wo2
        in_maps.append(m)
    return in_maps


def kernel(**inputs):
    if "nc" not in _CACHE:
        _CACHE["nc"] = _build_nc()
    nc = _CACHE["nc"]
    in_maps = _prep_in_maps(inputs)
    res = run_bass_kernel_spmd(nc, in_maps, core_ids=list(range(NCORES)))
    y = np.zeros((B, S, F), np.complex64)
    for core in range(NCORES):
        b = core // TPG
        y[b] += np.asarray(res.results[core]["yr"], np.float32)
        y[b] += 1j * np.asarray(res.results[core]["yi"], np.float32)
    return y


# revision 45
# speedup vs baseline: 1.0602x; 1.0602x over previous
"""ComplexCrossAttention Trainium2 kernel: 8 cores = DP(batch=2) x TP(head-groups=4).

Each core (b = core//4, g = core%4) handles batch b and heads 4g..4g+3; the
host adds the four per-group partial Wo outputs (the hint's all-reduce).

All matmul operands are fp16 (same 1 col/cycle PE rate as bf16 on TRN2, 3 extra
mantissa bits of precision; fp8 DoubleRow measured at only 2x-per-pass here, so
residual-fp8 loses to 16-bit).  Complex arithmetic is folded into the matmul
contraction: the j=0/j=1 planes of each packed operand carry the (w1, x_r) and
(w2, x_i) terms of  [real;imag] = w1^T x_r + w2^T x_i,  with w1 = [Wr|Wi],
w2 = [-Wi|Wr] column blocks per head.

Schedule: K proj, V proj, then per q-tile qt: attention(qt) is followed by
Q proj(qt+1) BEFORE O proj(qt), so the projection fills the softmax-tail
latency (dn -> reciprocal -> broadcast -> OT) of the last head.

Scores live transposed [k, q] so the mask is a per-partition activation bias.
Softmax denominators: e-tiles tree-summed on DVE (7 adds), ONE ones-matmul
[128,8]^T @ esum -> dn[8,512], reciprocal_approx_fast on [8,512], and a K=8
0.125-matmul broadcasting 1/dn to [128,512] PSUM.  This replaces the previous
128 full-size dn matmuls and 16 x 3.4us full DVE reciprocals.

Outputs are written as fp16 partials (summed on host).  Bias terms are all
zero in this problem and are folded out.
"""

import numpy as np
import ml_dtypes

import concourse.bacc as bacc
import concourse.mybir as mybir
import concourse.tile as tile
from concourse.bass_utils import run_bass_kernel_spmd

F32 = mybir.dt.float32
F16 = mybir.dt.float16
NP16 = np.float16

B, S, Lc = 2, 2048, 1024
F, Dc, H = 1024, 768, 16
HD = 64
NCORES = 8
TPG = 4            # head-groups (TP degree per batch)
FS = F // TPG      # 256 features per core
HL = 4             # heads per core
NQ, QTS = 4, 512   # q tiles
NKT = 8            # k tiles of 128 (Lc)
NFIN = 8           # f_in 128-chunks (Q proj contraction, per component)
NDC = 6            # Dc 128-chunks (K/V proj contraction, per component)
SCALE = 1.0 / 8.0  # 1/sqrt(HD)

_CACHE = {}

EXP = mybir.ActivationFunctionType.Exp


def _build_nc():
    nc = bacc.Bacc()
    dt = mybir.dt

    d = {}
    for nm, shape in (
        ("x", [128, NQ, NFIN, 2, QTS]),
        ("wq", [128, NFIN, 2, 512]),
        ("ct", [128, NDC, 2, Lc]),
        ("wk", [128, NDC, 2, 512]),
        ("wv", [128, NDC, 2, 512]),
        ("wo1", [128, 2, 2, F]),
        ("wo2", [128, 2, 2, F]),
    ):
        d[nm] = nc.dram_tensor(nm, shape, dt.float16, kind="ExternalInput")
    d["mb"] = nc.dram_tensor("mb", [128, NKT], dt.float32, kind="ExternalInput")
    yr_d = nc.dram_tensor("yr", [S, F], dt.float16, kind="ExternalOutput")
    yi_d = nc.dram_tensor("yi", [S, F], dt.float16, kind="ExternalOutput")

    with tile.TileContext(nc) as tc:
        with (
            tc.tile_pool(name="res", bufs=1) as res,
            tc.tile_pool(name="qx", bufs=8) as qxp,       # QX per (qt,h)
            tc.tile_pool(name="ep", bufs=8) as ep,        # exp(scores)
            tc.tile_pool(name="esp", bufs=8) as esp,      # e tree sums
            tc.tile_pool(name="oth", bufs=2) as othp,     # OT per qt
            tc.tile_pool(name="rcs", bufs=2) as rcs,      # small rec tiles
            tc.tile_pool(name="rcb", bufs=2) as rcbp,     # rec broadcast sbuf
            tc.tile_pool(name="ys", bufs=3) as ys,        # y staging
            tc.tile_pool(name="pp", bufs=2, space="PSUM") as pp,    # projections
            tc.tile_pool(name="sp", bufs=3, space="PSUM") as spp,   # scores
            tc.tile_pool(name="ap", bufs=2, space="PSUM") as avp,   # attn @ V
            tc.tile_pool(name="dn", bufs=1, space="PSUM") as dnp,   # dn [8,512]
        ):
            def rtile(shape, dtype, tag):
                return res.tile(shape, dtype, tag=tag, name=tag)

            # ---- input tiles + DMA (3 queues; order = consumption order) --
            sb = {}

            def load(name, shape, queue):
                t = rtile(shape, F16, name)
                queue.dma_start(t[:], d[name][:])
                sb[name] = t

            # Aggregate DMA bandwidth is only ~140-150 GB/s/core, so the
            # schedule is arrival-order-critical.  Keep >=8KB/partition
            # descriptors (small strided descriptors measured ~5x slower).
            # x is q-tile-major so Q(qt) gates only on its own 2MB slice.
            ct = rtile([128, NDC, 2, Lc], F16, "ct")
            xt = rtile([128, NQ, NFIN, 2, QTS], F16, "x")
            wk = rtile([128, NDC, 2, 512], F16, "wk")
            wv = rtile([128, NDC, 2, 512], F16, "wv")
            nc.scalar.dma_start(wk[:, 0:3], d["wk"][:, 0:3])
            nc.scalar.dma_start(wv[:, 0:3], d["wv"][:, 0:3])
            nc.scalar.dma_start(wk[:, 3:6], d["wk"][:, 3:6])
            nc.scalar.dma_start(wv[:, 3:6], d["wv"][:, 3:6])
            sb["wk"], sb["wv"] = wk, wv
            for c in range(NDC):
                (nc.sync if c % 2 == 0 else nc.gpsimd).dma_start(
                    ct[:, c:c + 1], d["ct"][:, c:c + 1])
            sb["ct"] = ct
            nc.sync.dma_start(xt[:, 0, 0:4], d["x"][:, 0, 0:4])
            nc.gpsimd.dma_start(xt[:, 0, 4:8], d["x"][:, 0, 4:8])
            nc.gpsimd.dma_start(xt[:, 1], d["x"][:, 1])
            nc.sync.dma_start(xt[:, 2], d["x"][:, 2])
            nc.gpsimd.dma_start(xt[:, 3], d["x"][:, 3])
            sb["x"] = xt
            load("wq", [128, NFIN, 2, 512], nc.scalar)
            mb = rtile([128, NKT], F32, "mb")
            nc.scalar.dma_start(mb[:], d["mb"][:])
            load("wo1", [128, 2, 2, F], nc.scalar)
            load("wo2", [128, 2, 2, F], nc.scalar)

            ones_dn = rtile([128, 8], F16, "ones_dn")
            nc.vector.memset(ones_dn[:], 1.0)

            KX = {h: rtile([128, Lc], F16, f"kx{h}") for h in range(HL)}
            Vsb = {kt: rtile([128, 512], F16, f"v{kt}") for kt in range(NKT)}

            def proj(ps, wname, xname, nch, lsl, rsl):
                last = nch * 2 - 1
                for c in range(nch):
                    for j in range(2):
                        nc.tensor.matmul(
                            ps, sb[wname][:, c, j, lsl], sb[xname][:, c, j, rsl],
                            start=(c == 0 and j == 0), stop=(c * 2 + j == last),
                        )

            def q_proj(qt):
                qx = {}
                for h in range(HL):
                    ps = pp.tile([128, 512], F32, tag="pp", name="pp")
                    hsl = slice(h * 128, (h + 1) * 128)
                    for c in range(NFIN):
                        for j in range(2):
                            nc.tensor.matmul(
                                ps[:], sb["wq"][:, c, j, hsl],
                                sb["x"][:, qt, c, j, :],
                                start=(c == 0 and j == 0),
                                stop=(c == NFIN - 1 and j == 1),
                            )
                    t = qxp.tile([128, 512], F16, tag="qx", name="qx")
                    nc.vector.tensor_copy(t[:], ps[:])
                    qx[h] = t
                return qx

            def attention(qt, qx):
                oth = othp.tile([128, HL, 512], F16, tag="oth", name="oth")
                for h in range(HL):
                    av = avp.tile([128, 512], F32, tag="av", name="av")
                    e_t, es_t = {}, []

                    def tree_add(a, b):
                        es = esp.tile([128, 512], F16, tag="es", name="es")
                        nc.vector.tensor_add(es[:], a[:], b[:])
                        return es

                    def scores_exp(kt):
                        ksl = slice(kt * 128, (kt + 1) * 128)
                        sps = spp.tile([128, 512], F32, tag="sp", name="sp")
                        nc.tensor.matmul(sps[:], KX[h][:, ksl], qx[h][:],
                                         start=True, stop=True)
                        e = ep.tile([128, 512], F16, tag="e", name="e")
                        nc.scalar.activation(e[:], sps[:], EXP,
                                             bias=mb[:, kt:kt + 1], scale=SCALE)
                        e_t[kt] = e
                        if kt % 2 == 1:
                            es_t.append(tree_add(e_t[kt - 1], e))
                            if kt % 4 == 3:
                                es_t.append(tree_add(es_t[-2], es_t[-1]))
                            if kt == 7:
                                es_t.append(tree_add(es_t[2], es_t[5]))

                    def av_mm(kt):
                        nc.tensor.matmul(av[:], Vsb[kt][:, h * 128:(h + 1) * 128],
                                         e_t[kt][:], start=(kt == 0),
                                         stop=(kt == NKT - 1))

                    # scores/exp lead av by one k-tile
                    for kt in range(NKT + 1):
                        if kt < NKT:
                            scores_exp(kt)
                        if kt > 0:
                            av_mm(kt - 1)

                    # dn bank is allocated full-height so pass A of the K/V
                    # projections can reuse it as an 8th accumulation group
                    dn = dnp.tile([128, 512], F32, tag="dn", name="dn")
                    nc.tensor.matmul(dn[0:8, :], ones_dn[:], es_t[-1][:],
                                     start=True, stop=True)
                    rec32 = rcs.tile([8, 512], F32, tag="r32", name="r32")
                    nc.vector.reciprocal_approx_fast(rec32[:], dn[0:8, :])
                    rec16 = rcs.tile([8, 512], F16, tag="r16", name="r16")
                    nc.vector.tensor_copy(rec16[:], rec32[:])
                    rbs = rcbp.tile([128, 512], F16, tag="rbs", name="rbs")
                    nc.gpsimd.partition_broadcast(rbs[:], rec16[0:1, :])
                    nc.vector.tensor_mul(oth[:, h, :], av[:], rbs[:])
                return oth

            def o_proj(qt, oth):
                for qi in range(4):
                    isl = slice(qi * 128, (qi + 1) * 128)
                    for kind, dram in (("wo1", yr_d), ("wo2", yi_d)):
                        st = ys.tile([128, F], F16, tag="y", name="y")
                        q0 = qt * QTS + qi * 128
                        last = qt == NQ - 1 and qi == 3
                        for fo in range(2):
                            fsl = slice(fo * 512, (fo + 1) * 512)
                            ps = pp.tile([128, 512], F32, tag="pp", name="pp")
                            for hp in range(2):
                                for j in range(2):
                                    nc.tensor.matmul(
                                        ps[:], oth[:, 2 * hp + j, isl],
                                        sb[kind][:, hp, j, fsl],
                                        start=(hp == 0 and j == 0),
                                        stop=(hp == 1 and j == 1),
                                    )
                            if (qi + fo) % 2 == 0:
                                nc.scalar.copy(st[:, fsl], ps[:])
                            else:
                                nc.vector.tensor_copy(st[:, fsl], ps[:])
                            if last:
                                # final row: per-half DMAs on both queues so
                                # the tail transfer halves and parallelizes
                                (nc.sync if fo == 0 else nc.gpsimd).dma_start(
                                    dram[q0:q0 + 128, fsl], st[:, fsl])
                        if not last:
                            (nc.sync if kind == "wo1" else nc.gpsimd).dma_start(
                                dram[q0:q0 + 128, :], st[:])

            # ---- K + V projections, two passes ---------------------------
            # Pass A: 3 K-groups + 4 V-groups held in 7 open PSUM banks,
            # chunk-outer, so matmuls consume each ct chunk DMA as it lands.
            # Pass B: the remaining 5 K + 4 V groups at full speed once ct
            # is resident.  This hides nearly all of the ct transfer.
            def k_mm(ps, kq, h, c, j, start, stop):
                nc.tensor.matmul(
                    ps, sb["wk"][:, c, j, h * 128:(h + 1) * 128],
                    sb["ct"][:, c, j, kq * 512:(kq + 1) * 512],
                    start=start, stop=stop)

            def v_mm(ps, kt, c, j, start, stop):
                nc.tensor.matmul(
                    ps, sb["ct"][:, c, j, kt * 128:(kt + 1) * 128],
                    sb["wv"][:, c, j, :], start=start, stop=stop)

            ka = [(0, 0), (0, 1), (0, 2)]
            va = [0, 1, 2, 3, 4]
            apools = [(pp, "pp"), (pp, "pp"), (spp, "sp"), (spp, "sp"),
                      (spp, "sp"), (avp, "av"), (avp, "av"), (dnp, "dn")]
            aps = {}
            for (pool, tag), key in zip(apools, ka + va):
                aps[key] = pool.tile([128, 512], F32, tag=tag, name="kv")
            for c in range(NDC):
                for j in range(2):
                    first, last = (c == 0 and j == 0), (c == NDC - 1 and j == 1)
                    for kq, h in ka:
                        k_mm(aps[(kq, h)][:], kq, h, c, j, first, last)
                    for kt in va:
                        v_mm(aps[kt][:], kt, c, j, first, last)
            for kq, h in ka:
                nc.vector.tensor_copy(KX[h][:, kq * 512:(kq + 1) * 512],
                                      aps[(kq, h)][:])
            for kt in va:
                nc.vector.tensor_copy(Vsb[kt][:], aps[kt][:])

            for kq, h in [(0, 3), (1, 0), (1, 1), (1, 2), (1, 3)]:
                ps = pp.tile([128, 512], F32, tag="pp", name="pp")
                for c in range(NDC):
                    for j in range(2):
                        k_mm(ps[:], kq, h, c, j, c == 0 and j == 0,
                             c == NDC - 1 and j == 1)
                nc.vector.tensor_copy(KX[h][:, kq * 512:(kq + 1) * 512], ps[:])
            for kt in range(5, NKT):
                ps = pp.tile([128, 512], F32, tag="pp", name="pp")
                for c in range(NDC):
                    for j in range(2):
                        v_mm(ps[:], kt, c, j, c == 0 and j == 0,
                             c == NDC - 1 and j == 1)
                nc.vector.tensor_copy(Vsb[kt][:], ps[:])

            # ---- pipelined per-q-tile: att(qt) -> Q(qt+1) -> O(qt) --------
            qx = q_proj(0)
            pend = None  # (qt, oth) awaiting O projection
            for qt in range(NQ):
                oth = attention(qt, qx)
                if qt + 1 < NQ:
                    qx = q_proj(qt + 1)
                if pend is not None:
                    o_proj(*pend)
                pend = (qt, oth)
            o_proj(*pend)

    nc.compile()
    return nc


def _prep_in_maps(inputs):
    f32 = np.float32
    x_r, x_i = np.asarray(inputs["x_r"], f32), np.asarray(inputs["x_i"], f32)
    ctx_r, ctx_i = np.asarray(inputs["ctx_r"], f32), np.asarray(inputs["ctx_i"], f32)
    mask = np.asarray(inputs["mask"], f32)
    W = {k: np.asarray(inputs[k], f32) for k in
         ("Wqr", "Wqi", "Wkr", "Wki", "Wvr", "Wvi", "Wor", "Woi")}

    def pack_moving(ar, ai, nch, n):
        """[n, nch*128] pair -> [128, nch, 2, n] fp16."""
        out = np.empty((128, nch, 2, n), NP16)
        out[:, :, 0, :] = ar.T.reshape(nch, 128, n).transpose(1, 0, 2)
        out[:, :, 1, :] = ai.T.reshape(nch, 128, n).transpose(1, 0, 2)
        return out

    per_batch = {}
    for b in range(B):
        xp = pack_moving(x_r[b], x_i[b], NFIN, S)  # [128, 8, 2, 2048]
        xp = np.ascontiguousarray(
            xp.reshape(128, NFIN, 2, NQ, QTS).transpose(0, 3, 1, 2, 4))
        per_batch[b] = {
            "x": xp,
            "ct": pack_moving(ctx_r[b], ctx_i[b], NDC, Lc),
            "mb": np.ascontiguousarray(
                ((1.0 - mask[b]) * -1e9).astype(f32).reshape(NKT, 128).T),
        }

    def merge_cols(Wr, Wi, g):
        """[Din, F] pair -> w1 = [Wr_h|Wi_h], w2 = [-Wi_h|Wr_h] col-blocks."""
        din = Wr.shape[0]
        w1 = np.empty((din, HL * 128), f32)
        w2 = np.empty((din, HL * 128), f32)
        for h in range(HL):
            cs = slice(g * FS + h * HD, g * FS + (h + 1) * HD)
            w1[:, h * 128:h * 128 + 64] = Wr[:, cs]
            w1[:, h * 128 + 64:(h + 1) * 128] = Wi[:, cs]
            w2[:, h * 128:h * 128 + 64] = -Wi[:, cs]
            w2[:, h * 128 + 64:(h + 1) * 128] = Wr[:, cs]
        return w1, w2

    def pack_w(w1, w2, nch):
        out = np.empty((128, nch, 2, 512), NP16)
        out[:, :, 0, :] = w1.reshape(nch, 128, 512).transpose(1, 0, 2)
        out[:, :, 1, :] = w2.reshape(nch, 128, 512).transpose(1, 0, 2)
        return out

    in_maps = []
    for core in range(NCORES):
        b, g = core // TPG, core % TPG
        m = dict(per_batch[b])
        for pre, wr, wi, nch in (("wq", "Wqr", "Wqi", NFIN),
                                 ("wk", "Wkr", "Wki", NDC),
                                 ("wv", "Wvr", "Wvi", NDC)):
            m[pre] = pack_w(*merge_cols(W[wr], W[wi], g), nch)
        # Wo rows in the merged [out_r(64); out_i(64)] layout, head-pair packed
        wo1 = np.empty((128, 2, 2, F), NP16)
        wo2 = np.empty((128, 2, 2, F), NP16)
        for h in range(HL):
            rs = slice(g * FS + h * HD, g * FS + (h + 1) * HD)
            hp, j = h // 2, h % 2
            wo1[:64, hp, j, :] = W["Wor"][rs]
            wo1[64:, hp, j, :] = -W["Woi"][rs]
            wo2[:64, hp, j, :] = W["Woi"][rs]
            wo2[64:, hp, j, :] = W["Wor"][rs]
        m["wo1"], m["wo2"] = wo1, wo2
        in_maps.append(m)
    return in_maps


def kernel(**inputs):
    if "nc" not in _CACHE:
        _CACHE["nc"] = _build_nc()
    nc = _CACHE["nc"]
    in_maps = _prep_in_maps(inputs)
    res = run_bass_kernel_spmd(nc, in_maps, core_ids=list(range(NCORES)))
    y = np.zeros((B, S, F), np.complex64)
    for core in range(NCORES):
        b = core // TPG
        y[b] += np.asarray(res.results[core]["yr"], np.float32)
        y[b] += 1j * np.asarray(res.results[core]["yi"], np.float32)
    return y
